# revision 8
# baseline (speedup 1.0000x reference)
"""Trainium2 Bass kernel for nn_Attention_61443802137307 (v2).

Design (per core; data-parallel over batch, 32 batches/core, 4 pairs of 8):
  - Token slots per pair: 1280 = 8 x 128 "kA blocks" (batch b keys/queries
    0:128) + 2 x 128 "kB blocks" (per quad: 4 x (16 key-tail + 16 zero pad)).
    Host reorders/pads x, x+topo into slot order.
  - Q/K projections in fp8e4m3 with DoubleRow matmuls (4x PE throughput);
    results evacuated to bf16, scores in plain bf16 (fp8 evac
    requantization was the error budget's largest term). Weights
    pre-scaled by 32; combined descale folded into the exp scale. Softmax
    turns relative score error into tiny absolute error.
  - V projection / AV / output projection in bf16 (accuracy-critical).
  - AV accumulates kA (contract 128 @ base 0) then the quad kB block
    (contract 32 @ base 32c, explicit tile_position, skip_group_check).
  - Softmax denominators via ones-column of V; reciprocal on DVE; per-head
    staging to partition 0 + gpsimd partition_broadcast (only reads
    partition 0); applied by 4x-mode DVE multiplies that also relayout
    heads for the projection.
  - DMA count kept small (~15/pair) to keep HWDGE/SP.SEQ off the critical
    path (the v1 kernel was SP.SEQ-bound at 353us).
"""
import numpy as np
import ml_dtypes

import concourse.bass as bass
import concourse.tile as tile
import concourse.mybir as mybir
from concourse import bacc
from concourse.bass_utils import run_bass_kernel_spmd
from contextlib import ExitStack

F32 = mybir.dt.float32
BF16 = mybir.dt.bfloat16
FP8 = mybir.dt.float8e4
AF = mybir.ActivationFunctionType
DR = mybir.MatmulPerfMode.DoubleRow

BF16NP = ml_dtypes.bfloat16
FP8NP = ml_dtypes.float8_e4m3

B, N, D = 256, 144, 512
H, HD = 8, 64
SCALE = HD ** -0.5
WS = 32.0                     # fp8 weight pre-scale
ESCALE = SCALE / (WS * WS)    # folded into exp
N_CORES = 8
BPC = B // N_CORES            # 32 batches per core
TOK = BPC * N                 # 4608 real tokens per core
GB = 8                        # batches per pair-group
NG = BPC // GB                # 4 pair-groups
GTOK = GB * N                 # 1152 real tokens per group
SLOTS = 1280                  # padded slots per group (8x128 kA + 2x128 kB)

_CACHE = {}


def _slot_map():
    """slot index -> source token index within a group (-1 for pads)."""
    m = np.full(SLOTS, -1, dtype=np.int64)
    for b in range(GB):
        m[128 * b:128 * b + 128] = N * b + np.arange(128)
    for cg in range(2):
        for c in range(4):
            b = 4 * cg + c
            s0 = 1024 + 128 * cg + 32 * c
            m[s0:s0 + 16] = N * b + 128 + np.arange(16)
    return m


def build():
    nc = bacc.Bacc("TRN2", target_bir_lowering=False, debug=False,
                   num_devices=N_CORES)

    NSL = NG * SLOTS  # 5120 padded slots per core
    xq_d = nc.dram_tensor("xq", [128, 2, 2, NSL], FP8, kind="ExternalInput").ap()
    xkq_d = nc.dram_tensor("xkq", [128, 2, 2, NSL], FP8,
                           kind="ExternalInput").ap()
    xkv_d = nc.dram_tensor("xkv", [128, 4, NSL], BF16,
                           kind="ExternalInput").ap()
    qw_d = nc.dram_tensor("qw", [2, 128, 2, 4, 128], FP8,
                          kind="ExternalInput").ap()
    kw_d = nc.dram_tensor("kw", [2, 128, 2, 4, 128], FP8,
                          kind="ExternalInput").ap()
    vw_d = nc.dram_tensor("vw", [4, 128, 512], BF16, kind="ExternalInput").ap()
    pw_d = nc.dram_tensor("pw", [4, 128, 512], BF16, kind="ExternalInput").ap()
    pb_d = nc.dram_tensor("pb", [D], F32, kind="ExternalInput").ap()
    out = nc.dram_tensor("out", [TOK, D], F32, kind="ExternalOutput").ap()
    import os
    dbg = os.environ.get("K_DEBUG") == "1"
    if dbg:
        dbg_qfm = nc.dram_tensor("dbg_qfm", [128, 4, SLOTS], BF16,
                                 kind="ExternalOutput").ap()
        dbg_kfm = nc.dram_tensor("dbg_kfm", [128, 4, SLOTS], BF16,
                                 kind="ExternalOutput").ap()
        dbg_vA0 = nc.dram_tensor("dbg_vA0", [128, H, 65], BF16,
                                 kind="ExternalOutput").ap()
        dbg_vB = nc.dram_tensor("dbg_vB", [128, 2, H, 65], BF16,
                                kind="ExternalOutput").ap()
        dbg_e1 = nc.dram_tensor("dbg_e1", [128, 4, 2, 144], BF16,
                                kind="ExternalOutput").ap()
        dbg_au = nc.dram_tensor("dbg_au", [65, H, GTOK], BF16,
                                kind="ExternalOutput").ap()
        dbg_bc = nc.dram_tensor("dbg_bc", [64, H, GTOK], BF16,
                                kind="ExternalOutput").ap()
        dbg_ac = nc.dram_tensor("dbg_ac", [128, 4, GTOK], BF16,
                                kind="ExternalOutput").ap()
        dbg_sums = nc.dram_tensor("dbg_sums", [128, 2, GTOK], BF16,
                                  kind="ExternalOutput").ap()
        dbg_rc = nc.dram_tensor("dbg_rc", [128, 2, GTOK], BF16,
                                kind="ExternalOutput").ap()

    with tile.TileContext(nc) as tc, ExitStack() as ctx:
        singles = ctx.enter_context(tc.tile_pool(name="singles", bufs=1))
        xpool = ctx.enter_context(tc.tile_pool(name="xpool", bufs=2))
        qkpool = ctx.enter_context(tc.tile_pool(name="qkpool", bufs=2))
        vpool = ctx.enter_context(tc.tile_pool(name="vpool", bufs=2))
        epool = ctx.enter_context(tc.tile_pool(name="epool", bufs=4))
        aupool = ctx.enter_context(tc.tile_pool(name="aupool", bufs=2))
        acpool = ctx.enter_context(tc.tile_pool(name="acpool", bufs=1))
        spool = ctx.enter_context(tc.tile_pool(name="spool", bufs=3))
        bcpool = ctx.enter_context(tc.tile_pool(name="bcpool", bufs=3))
        opool = ctx.enter_context(tc.tile_pool(name="opool", bufs=3))
        ps_pj = ctx.enter_context(tc.tile_pool(name="ps_pj", bufs=3,
                                               space="PSUM"))
        ps_sc = ctx.enter_context(tc.tile_pool(name="ps_sc", bufs=2,
                                               space="PSUM"))
        ps_av = ctx.enter_context(tc.tile_pool(name="ps_av", bufs=1,
                                               space="PSUM"))

        # --- persistent weights ---
        qw_sb = singles.tile([128, 2, 2, 4, 128], FP8)
        kw_sb = singles.tile([128, 2, 2, 4, 128], FP8)
        for kc in range(2):
            nc.sync.dma_start(qw_sb[:, kc], qw_d[kc])
            nc.sync.dma_start(kw_sb[:, kc], kw_d[kc])
        vw_sb = singles.tile([128, 4, 512], BF16)
        pw_sb = singles.tile([128, 4, 512], BF16)
        for kc in range(4):
            nc.sync.dma_start(vw_sb[:, kc], vw_d[kc])
            nc.sync.dma_start(pw_sb[:, kc], pw_d[kc])
        bias_bc = singles.tile([128, D], F32)
        bias_src = bass.AP(tensor=pb_d.tensor, offset=0, ap=[[0, 128], [1, D]])
        nc.gpsimd.dma_start(out=bias_bc[:], in_=bias_src)

        QCH = (512, 512, 256)  # slot chunks for Q/K projection psum tiles

        def phase_A_load(g):
            s0 = g * SLOTS
            xq = xpool.tile([128, 2, 2, SLOTS], FP8, tag="xq", name=f"xq{g}")
            xkq = xpool.tile([128, 2, 2, SLOTS], FP8, tag="xkq", name=f"xkq{g}")
            xkv = xpool.tile([128, 4, SLOTS], BF16, tag="xkv", name=f"xkv{g}")
            nc.sync.dma_start(xq[:], xq_d[:, :, :, s0:s0 + SLOTS])
            nc.sync.dma_start(xkq[:], xkq_d[:, :, :, s0:s0 + SLOTS])
            nc.sync.dma_start(xkv[:], xkv_d[:, :, s0:s0 + SLOTS])
            q_fm = qkpool.tile([128, 4, SLOTS], BF16, tag="qfm",
                               name=f"qfm{g}")
            k_fm = qkpool.tile([128, 4, SLOTS], BF16, tag="kfm",
                               name=f"kfm{g}")
            vA = [vpool.tile([128, H, 65], BF16, tag=f"vA{i}",
                             name=f"vA{g}_{i}") for i in range(GB)]
            vB = vpool.tile([128, 2, H, 65], BF16, tag="vB", name=f"vB{g}")
            return (xq, xkq, xkv, q_fm, k_fm, vA, vB)

        def phase_A_part(g, i, st):
            xq, xkq, xkv, q_fm, k_fm, vA, vB = st
            # i 0..7. Schedule: Q/K proj (g2,t) combos on i=0..3 (3 chunks
            # each for both q and k), V blocks on i spread out.
            if i < 4:
                hp = i
                for (xsrc, w_sb, dst, ev) in ((xq, qw_sb, q_fm, "q"),
                                              (xkq, kw_sb, k_fm, "k")):
                    c0 = 0
                    for ch in QCH:
                        p = ps_pj.tile([128, 512], F32, tag="pj",
                                       name=f"pj{ev}{g}_{i}_{c0}")
                        for kc in range(2):
                            nc.tensor.matmul(
                                p[:, 0:ch],
                                w_sb[:, kc, :, hp, :],
                                xsrc[:, kc, :, c0:c0 + ch],
                                start=(kc == 0), stop=(kc == 1),
                                perf_mode=DR)
                        if ev == "q":
                            nc.vector.tensor_copy(
                                dst[:, hp, c0:c0 + ch], p[:, 0:ch])
                        else:
                            nc.scalar.copy(
                                dst[:, hp, c0:c0 + ch], p[:, 0:ch])
                        c0 += ch
            # V: 10 blocks over i: i0:blk0, i1:blk1, i2:b2,b3, i3:b4,
            # i4:b5, i5:b6, i6:b7,B8, i7:B9
            vsched = ([[0], [1], [2, 3], [4], [5], [6], [7, 8], [9]])[i]
            for blk in vsched:
                p = ps_pj.tile([128, 512], F32, tag="pj", name=f"pjv{g}_{blk}")
                for kc in range(4):
                    nc.tensor.matmul(
                        p[:],
                        xkv[:, kc, 128 * blk:128 * blk + 128],
                        vw_sb[:, kc, :],
                        start=(kc == 0), stop=(kc == 3))
                pv = p[:].rearrange("p (h d) -> p h d", h=H)
                if blk < 8:
                    nc.vector.tensor_copy(vA[blk][:, :, 0:64], pv)
                    nc.gpsimd.memset(vA[blk][:, :, 64:65], 1.0)
                else:
                    cg = blk - 8
                    nc.vector.tensor_copy(vB[:, cg, :, 0:64], pv)
            if i == 7:
                nc.gpsimd.memset(vB[:, :, :, 64:65], 0.0)
                for c in range(4):
                    nc.gpsimd.memset(vB[32 * c:32 * c + 16, :, :, 64:65], 1.0)
                if dbg and g == 0:
                    nc.sync.dma_start(dbg_qfm, q_fm[:])
                    nc.sync.dma_start(dbg_kfm, k_fm[:])
                    nc.sync.dma_start(dbg_vA0, vA[0][:])
                    nc.sync.dma_start(dbg_vB, vB[:])

        def phase_B_start(g):
            return aupool.tile([65, H, GTOK], BF16, tag="au", name=f"au{g}")

        def phase_B_part(g, b, st, attn_u):
            _, _, _, q_fm, k_fm, vA, vB = st
            cg, c = b // 4, b % 4
            for hg2 in range(4):
                sc = ps_sc.tile([128, 2, 2, 256], F32, tag="sc",
                                name=f"sc{g}_{b}_{hg2}")
                for j in range(2):
                    h = 2 * hg2 + j
                    r0 = 64 * j
                    lhsA = k_fm[r0:r0 + 64, hg2, 128 * b:128 * b + 128]
                    lhsB = k_fm[r0:r0 + 64, hg2,
                                1024 + 128 * cg:1024 + 128 * cg + 128]
                    rq = q_fm[r0:r0 + 64, hg2, :]
                    rhsA = rq[:, 128 * b:128 * b + 128]
                    qb0 = 1024 + 128 * cg + 32 * c
                    rhsB = rq[:, qb0:qb0 + 16]
                    nc.tensor.matmul(sc[:, j, 0, 0:128], lhsA, rhsA,
                                     start=True, stop=True)
                    nc.tensor.matmul(sc[:, j, 0, 128:144], lhsA, rhsB,
                                     start=True, stop=True)
                    nc.tensor.matmul(sc[:, j, 1, 0:128], lhsB, rhsA,
                                     start=True, stop=True)
                    nc.tensor.matmul(sc[:, j, 1, 128:144], lhsB, rhsB,
                                     start=True, stop=True)
                e1 = epool.tile([128, 2, 2, 144], BF16, tag="e1",
                                name=f"e1_{g}{b}{hg2}")
                nc.scalar.activation(e1[:], sc[:, :, :, 0:144], AF.Exp,
                                     scale=ESCALE)
                av = ps_av.tile([65, 512], F32, tag="av",
                                name=f"av{g}_{b}_{hg2}")
                for j in range(2):
                    h = 2 * hg2 + j
                    nc.tensor.matmul(
                        av[:, 256 * j:256 * j + 144],
                        vA[b][:, h, :], e1[:, j, 0, :],
                        start=True, stop=True)
                    nc.tensor.matmul(
                        av[:, 256 * j:256 * j + 144],
                        vB[32 * c:32 * c + 32, cg, h, :],
                        e1[32 * c:32 * c + 32, j, 1, :],
                        start=False, stop=True, skip_group_check=True,
                        tile_position=(32 * c, 0))
                avv = av[:].rearrange("p (j x) -> p j x", j=2)[:, :, 0:144]
                dst = attn_u[:, 2 * hg2:2 * hg2 + 2, N * b:N * b + 144]
                if hg2 % 2 == 0:
                    nc.scalar.copy(dst, avv)
                else:
                    nc.vector.tensor_copy(dst, avv)

        def phase_C_start(g, attn_u):
            # partition_broadcast only reads from partition 0, so stage each
            # head's reciprocal into its own single-partition tile first
            sums = spool.tile([8, GTOK], BF16, tag="sums", name=f"sums{g}")
            nc.sync.dma_start(sums[:], attn_u[64:65, :, :])
            recip = spool.tile([8, GTOK], BF16, tag="recip", name=f"rc{g}")
            with nc.allow_low_precision(reason="softmax recip in bf16"):
                nc.vector.reciprocal(recip[:], sums[:])
            attn_c = acpool.tile([128, 4, GTOK], BF16, tag="ac", name=f"ac{g}")
            if dbg and g == 0:
                nc.sync.dma_start(dbg_au, attn_u[:])
            return recip, attn_c

        def phase_C_part(g, i, attn_u, recip, attn_c):
            g0 = g * GTOK
            # per head: stage recip to partition 0, broadcast, multiply
            if i < 4:
                for h in (2 * i, 2 * i + 1):
                    fc, r0 = h // 2, (h % 2) * 64
                    rch = spool.tile([1, GTOK], BF16, tag="rch",
                                     name=f"rch{g}_{h}")
                    nc.sync.dma_start(rch[:], recip[h:h + 1, :])
                    bch = bcpool.tile([64, GTOK], BF16, tag="bch",
                                      name=f"bch{g}_{h}")
                    nc.gpsimd.partition_broadcast(bch[:], rch[0:1, :])
                    nc.vector.tensor_mul(
                        attn_c[r0:r0 + 64, fc, :],
                        attn_u[0:64, h, :], bch[:])
            if dbg and g == 0 and i == 7:
                nc.sync.dma_start(dbg_ac, attn_c[:])
            # proj tiles only after ALL 16 bc-muls (i=0..3) have been emitted
            tsched = ([[], [], [], [0, 1], [2, 3], [4, 5], [6, 7], [8]])[i]
            for t in tsched:
                p = ps_pj.tile([128, 512], F32, tag="pj", name=f"pjP{g}_{t}")
                for fc in range(4):
                    nc.tensor.matmul(
                        p[:],
                        attn_c[:, fc, 128 * t:128 * t + 128],
                        pw_sb[:, fc, :],
                        start=(fc == 0), stop=(fc == 3))
                o_sb = opool.tile([128, D], F32, tag="osb", name=f"osb{g}_{t}")
                nc.vector.tensor_add(o_sb[:], p[:], bias_bc[:])
                nc.sync.dma_start(
                    out[g0 + 128 * t:g0 + 128 * t + 128, :], o_sb[:])

        state = {}
        au = {}
        cst = {}
        for g in range(NG + 2):
            if g < NG:
                state[g] = phase_A_load(g)
            if 0 <= g - 1 < NG:
                au[g - 1] = phase_B_start(g - 1)
            if 0 <= g - 2 < NG:
                cst[g - 2] = phase_C_start(g - 2, au[g - 2])
            for i in range(GB):
                if g < NG:
                    phase_A_part(g, i, state[g])
                if 0 <= g - 2 < NG:
                    phase_C_part(g - 2, i, au[g - 2], *cst[g - 2])
                if 0 <= g - 1 < NG:
                    phase_B_part(g - 1, i, state[g - 1], au[g - 1])
            state.pop(g - 2, None)
            au.pop(g - 3, None)
            cst.pop(g - 3, None)

    nc.compile()
    return nc


def _get_nc():
    if "nc" not in _CACHE:
        _CACHE["nc"] = build()
    return _CACHE["nc"]


def _prep_weights(kv_w, q_w, proj_w, proj_b):
    kw = np.ascontiguousarray(kv_w[0:D] * WS)      # [D out, D in]
    vw = np.ascontiguousarray(kv_w[D:2 * D])
    # fp8 DoubleRow layout: [kc, kpart, kt, g, t, m]; in-feature =
    # 256*kc + 128*kt + kpart; out column m of mm (g,t): head 4*g2 + m//32,
    # feat 32*t + m%32
    def qk_arrange(w):
        a = np.zeros((2, 128, 2, 4, 128), dtype=np.float32)
        fin = np.arange(D)
        kc, rem = fin // 256, fin % 256
        kt, kp = rem // 128, rem % 128
        for hp in range(4):
            for m in range(128):
                head = 2 * hp + m // 64
                feat = m % 64
                o = head * HD + feat
                a[kc, kp, kt, hp, m] = w[o, fin]
        return a.astype(FP8NP)

    qw = qk_arrange(np.asarray(q_w) * WS)
    kw2 = qk_arrange(kw)
    # vw/pw: [kc, kpart, out]: lhsT = x/attn feature-major chunks
    vw2 = np.ascontiguousarray(vw.T).reshape(4, 128, 512).astype(BF16NP)
    pw2 = np.ascontiguousarray(np.asarray(proj_w).T).reshape(
        4, 128, 512).astype(BF16NP)
    return qw, kw2, vw2, pw2, np.asarray(proj_b, dtype=np.float32)


def _prep_core_inputs(x, xk, qw, kw, vw, pw, pb, smap):
    """x, xk: [BPC, N, D] f32 for one core -> slot-ordered fp8/bf16 streams."""
    nsl = NG * SLOTS
    xs = np.zeros((nsl, D), dtype=np.float32)
    xks = np.zeros((nsl, D), dtype=np.float32)
    xf = x.reshape(BPC * N, D)
    xkf = xk.reshape(BPC * N, D)
    for g in range(NG):
        m = smap  # per-group slot -> group-token
        valid = m >= 0
        gt = m[valid] + g * GTOK
        xs[g * SLOTS:(g + 1) * SLOTS][valid] = xf[gt]
        xks[g * SLOTS:(g + 1) * SLOTS][valid] = xkf[gt]

    def fp8_fm(a):  # [nsl, D] -> [128, 2, 2, nsl] (kpart, kc, kt)
        t = np.ascontiguousarray(a.T)                  # [D, nsl]
        t = t.reshape(2, 2, 128, nsl)                  # [kc, kt, kp, nsl]
        return np.ascontiguousarray(
            t.transpose(2, 0, 1, 3)).astype(FP8NP)     # [kp, kc, kt, nsl]

    def bf16_fm(a):  # [nsl, D] -> [128, 4, nsl]
        t = np.ascontiguousarray(a.T).reshape(4, 128, nsl)
        return np.ascontiguousarray(t.transpose(1, 0, 2)).astype(BF16NP)

    return {
        "xq": fp8_fm(xs), "xkq": fp8_fm(xks), "xkv": bf16_fm(xks),
        "qw": qw, "kw": kw, "vw": vw, "pw": pw, "pb": pb,
    }


def kernel(x, topo_all_fea, kv_w, q_w, proj_w, proj_b, is_end):
    x = np.asarray(x, dtype=np.float32)
    topo = np.asarray(topo_all_fea, dtype=np.float32)
    kv_w = np.asarray(kv_w, dtype=np.float32)
    q_w = np.asarray(q_w, dtype=np.float32)
    proj_w = np.asarray(proj_w, dtype=np.float32)
    proj_b = np.asarray(proj_b, dtype=np.float32)
    end = bool(np.asarray(is_end).item()) if not isinstance(is_end, bool) \
        else is_end

    xk = x + topo if end else x
    qw, kw, vw, pw, pb = _prep_weights(kv_w, q_w, proj_w, proj_b)
    smap = _slot_map()

    nc = _get_nc()
    in_maps = [
        _prep_core_inputs(x[c * BPC:(c + 1) * BPC],
                          xk[c * BPC:(c + 1) * BPC],
                          qw, kw, vw, pw, pb, smap)
        for c in range(N_CORES)
    ]
    res = run_bass_kernel_spmd(nc, in_maps, core_ids=list(range(N_CORES)))
    outs = [res.results[c]["out"].reshape(BPC, N, D) for c in range(N_CORES)]
    return np.concatenate(outs, axis=0)
